# revision 10
# baseline (speedup 1.0000x reference)
"""GQA causal attention (Llama prefill) on 8 TRN2 NeuronCores.

Sharding: tensor-parallel over KV heads (4-way: 2 KV heads -> 8 Q heads per
core) x data-parallel over batch (2-way).  Core i handles batch i//4 and KV
head pair i%4.  Each core computes its 8 heads' attention and a partial
o_proj ([S, H] contribution from its 1024 columns of attn output); the host
sums the 4 partials per batch element.

Layout strategy (everything transposed so no transposes are needed after the
initial hidden -> hidden^T step):
  hidT[h, t]  (PE transpose of streamed hidden tiles)
  qT[d, t] = w_q^T-slice . hidT   (w_q tile as stationary)
  kT[d, t]   likewise;  v[t, d] natural (hidT tile as stationary)
  S^T[tk, tq] = kT-tile^T . qT    (softmax sums land on matmul-with-ones)
  P^T = exp(S^T * scale) * causal01mask
  attnT[d, tq] += v-tile^T . P^T  ;  l[1, tq] += ones^T . P^T
  attnT *= broadcast(1/l)
  out[tq, e] += attnT-tile^T . w_o  (attnT tile as stationary)
Matmuls run as float32r (full-rate fp32 PE mode; storage stays fp32).
"""

import numpy as np
from contextlib import ExitStack

import concourse.bass as bass
import concourse.tile as tile
from concourse import mybir, bacc
from concourse.masks import make_identity

F32 = mybir.dt.float32
F32R = mybir.dt.float32r
P = 128

# full-problem config
NUM_HEADS = 32
NUM_KV_HEADS = 8
HEAD_DIM = 128
B, S_FULL, H_FULL = 2, 2048, 4096
TP = 4                     # kv-head-pair shards
DP = 2                     # batch shards
N_CORES = TP * DP


def _r(ap):
    return ap if ap.dtype == F32R else ap.bitcast(F32R)


def build_attention_nc(S=2048, H=4096, n_qh=8, n_kvh=2, D=128, CHUNK=512):
    """One core's program: hid [S,H] x wq [H,n_qh*D] x wk/wv [H,n_kvh*D]
    x wo [n_qh*D, H] -> partial out [S,H]."""
    DQ = n_qh * D
    DKV = n_kvh * D
    g = n_qh // n_kvh
    scale = float(D) ** -0.5
    n_ht = H // P              # h-tiles
    n_ch = S // CHUNK          # token chunks
    spc = CHUNK // P           # 128-subtiles per chunk
    n_tt = S // P              # t-tiles
    assert S % CHUNK == 0 and CHUNK % P == 0 and H % 512 == 0 and DKV <= 512

    nc = bacc.Bacc("TRN2", target_bir_lowering=False, debug=False)
    hid = nc.dram_tensor("hid", [S, H], F32, kind="ExternalInput").ap()
    wq = nc.dram_tensor("wq", [H, DQ], F32, kind="ExternalInput").ap()
    wk = nc.dram_tensor("wk", [H, DKV], F32, kind="ExternalInput").ap()
    wv = nc.dram_tensor("wv", [H, DKV], F32, kind="ExternalInput").ap()
    wo = nc.dram_tensor("wo", [DQ, H], F32, kind="ExternalInput").ap()
    out = nc.dram_tensor("out", [S, H], F32, kind="ExternalOutput").ap()

    # alternate psum->sbuf copies between ACT and DVE
    _cp_state = [0]

    def copy_ps(dst, src):
        _cp_state[0] ^= 1
        if _cp_state[0]:
            nc.scalar.copy(dst, src)
        else:
            nc.vector.tensor_copy(dst, src)

    with tile.TileContext(nc) as tc, ExitStack() as ctx:
        const = ctx.enter_context(tc.tile_pool(name="const", bufs=1))
        persist = ctx.enter_context(tc.tile_pool(name="persist", bufs=1))

        ident = const.tile([P, P], F32, name="ident")
        make_identity(nc, ident)
        ones_f32 = const.tile([P, 1], F32, name="ones_f32")
        nc.vector.memset(ones_f32, 1.0)
        ones_col = const.tile([P, 1], F32R, name="ones_col")
        nc.vector.tensor_copy(ones_col, ones_f32)
        # one sliding causal 0/1 mask covering all spc straddle positions:
        # mask_big[r, f] = 1.0 iff r <= f - (spc-1)*128; straddle position p
        # uses the slice starting at (spc-1-p)*128.
        mask_w = CHUNK + (spc - 1) * P
        mask_big = const.tile([P, mask_w], F32, name="mask_big")
        nc.vector.memset(mask_big, 1.0)
        nc.gpsimd.affine_select(
            out=mask_big, in_=mask_big,
            pattern=[[1, mask_w]], base=-(spc - 1) * P, channel_multiplier=-1,
            compare_op=mybir.AluOpType.is_ge, fill=0.0,
        )

        def mask_for(p_):
            off = (spc - 1 - p_) * P
            return mask_big[:, off:off + CHUNK]

        qT = [persist.tile([P, S], F32R, name=f"qT{d}", tag=f"qT{d}")
              for d in range(n_qh)]
        kT = [persist.tile([P, S], F32R, name=f"kT{d}", tag=f"kT{d}")
              for d in range(n_kvh)]
        v_sb = [persist.tile([P, DKV], F32R, name=f"v{j}", tag=f"v{j}")
                for j in range(n_tt)]
        # attnT[h] reuses qT[h-1]'s slot (dead once head h-1's scores are
        # done); attnT[0] gets its own slot.  Tile's tag aliasing inserts the
        # WAR dependency automatically.
        attnT = [persist.tile([P, S], F32R, name=f"attnT{d}",
                              tag=("attnT0" if d == 0 else f"qT{d - 1}"))
                 for d in range(n_qh)]

        # ---------------- phase 1: hidT + q/k/v projections ----------------
        with tc.tile_pool(name="hidT", bufs=1) as hidT_pool, \
             tc.tile_pool(name="p_in", bufs=3) as in_pool, \
             tc.tile_pool(name="p_w", bufs=3) as w_pool, \
             tc.tile_pool(name="tp_ps", bufs=2, space="PSUM") as tp_psum, \
             tc.tile_pool(name="pp_ps", bufs=4, space="PSUM") as pp_psum:
            for c in range(n_ch):
                # hidT[hi] = hidden[c-chunk, :]^T, one [128, CHUNK] tile per h-tile
                hidT = [hidT_pool.tile([P, CHUNK], F32R, name=f"hidT{hi}_{c}",
                                       tag=f"hidT{hi}") for hi in range(n_ht)]
                for hc in range(H // 512):
                    for ts_ in range(spc):
                        hin = in_pool.tile([P, 512], F32, name=f"hin_{c}_{hc}_{ts_}",
                                           tag="hid_in")
                        nc.gpsimd.dma_start(
                            hin, hid[c * CHUNK + ts_ * P: c * CHUNK + (ts_ + 1) * P,
                                     hc * 512:(hc + 1) * 512])
                        tp = tp_psum.tile([P, 512], F32, name=f"tp_{c}_{hc}_{ts_}",
                                          tag="tp")
                        for s4 in range(4):
                            hi = hc * 4 + s4
                            nc.tensor.transpose(
                                tp[:, s4 * P:(s4 + 1) * P],
                                hin[:, s4 * P:(s4 + 1) * P], ident)
                            copy_ps(hidT[hi][:, ts_ * P:(ts_ + 1) * P],
                                    tp[:, s4 * P:(s4 + 1) * P])

                # k^T projection: psum[d][128, CHUNK] accumulated over h-tiles
                kps = [pp_psum.tile([P, CHUNK], F32, name=f"kps{d}_{c}", tag="pp")
                       for d in range(n_kvh)]
                for hi in range(n_ht):
                    wkt = w_pool.tile([P, DKV], F32R, name=f"wk_{c}_{hi}", tag="wk_in")
                    nc.gpsimd.dma_start(wkt, wk[hi * P:(hi + 1) * P, :].bitcast(F32R))
                    for d in range(n_kvh):
                        nc.tensor.matmul(
                            kps[d], _r(wkt[:, d * P:(d + 1) * P]), _r(hidT[hi]),
                            start=(hi == 0), stop=(hi == n_ht - 1))
                for d in range(n_kvh):
                    copy_ps(kT[d][:, c * CHUNK:(c + 1) * CHUNK], kps[d])

                # v projection (natural layout): psum[tsub][128, DKV]
                vps = [pp_psum.tile([P, DKV], F32, name=f"vps{t}_{c}", tag="pp")
                       for t in range(spc)]
                for hi in range(n_ht):
                    wvt = w_pool.tile([P, DKV], F32R, name=f"wv_{c}_{hi}", tag="wv_in")
                    nc.gpsimd.dma_start(wvt, wv[hi * P:(hi + 1) * P, :].bitcast(F32R))
                    for t in range(spc):
                        nc.tensor.matmul(
                            vps[t], _r(hidT[hi][:, t * P:(t + 1) * P]), _r(wvt),
                            start=(hi == 0), stop=(hi == n_ht - 1))
                for t in range(spc):
                    copy_ps(v_sb[c * spc + t], vps[t])

                # q^T projection in groups of 2 d-tiles
                for grp in range(n_qh // 2):
                    qps = [pp_psum.tile([P, CHUNK], F32, name=f"qps{grp}_{d}_{c}",
                                        tag="pp") for d in range(2)]
                    for hi in range(n_ht):
                        wqt = w_pool.tile([P, 2 * P], F32R, name=f"wq_{c}_{grp}_{hi}",
                                          tag="wq_in")
                        nc.gpsimd.dma_start(
                            wqt, wq[hi * P:(hi + 1) * P,
                                    grp * 2 * P:(grp + 1) * 2 * P].bitcast(F32R))
                        for d in range(2):
                            nc.tensor.matmul(
                                qps[d], _r(wqt[:, d * P:(d + 1) * P]), _r(hidT[hi]),
                                start=(hi == 0), stop=(hi == n_ht - 1))
                    for d in range(2):
                        copy_ps(qT[grp * 2 + d][:, c * CHUNK:(c + 1) * CHUNK], qps[d])

        # ---------------- phase 2: attention ----------------
        with tc.tile_pool(name="s_ps", bufs=3, space="PSUM") as s_psum, \
             tc.tile_pool(name="pv_ps", bufs=2, space="PSUM") as pv_psum, \
             tc.tile_pool(name="l_ps", bufs=2, space="PSUM") as l_psum, \
             tc.tile_pool(name="pt", bufs=4) as pt_pool, \
             tc.tile_pool(name="att_sm", bufs=2) as sm_pool, \
             tc.tile_pool(name="att_rb", bufs=2) as rb_pool:
            for h in range(n_qh):
                kv = h // g
                for c in range(n_ch):
                    jmax = spc * (c + 1)
                    pvps = pv_psum.tile([P, CHUNK], F32, name=f"pv_{h}_{c}", tag="pv")
                    lps = l_psum.tile([1, CHUNK], F32, name=f"l_{h}_{c}", tag="l")
                    for j in range(jmax):
                        sps = s_psum.tile([P, CHUNK], F32, name=f"s_{h}_{c}_{j}",
                                          tag="s")
                        nc.tensor.matmul(
                            sps, _r(kT[kv][:, j * P:(j + 1) * P]),
                            _r(qT[h][:, c * CHUNK:(c + 1) * CHUNK]),
                            start=True, stop=True)
                        pt = pt_pool.tile([P, CHUNK], F32R, name=f"pt_{h}_{c}_{j}",
                                          tag="pt")
                        nc.scalar.activation(pt, sps,
                                             mybir.ActivationFunctionType.Exp,
                                             scale=scale)
                        p_ = j - spc * c
                        if p_ >= 0:
                            nc.vector.tensor_mul(pt, pt, mask_for(p_))
                        nc.tensor.matmul(pvps, _r(v_sb[j][:, kv * P:(kv + 1) * P]),
                                         _r(pt), start=(j == 0), stop=(j == jmax - 1))
                        nc.tensor.matmul(lps, _r(ones_col), _r(pt),
                                         start=(j == 0), stop=(j == jmax - 1))
                    at_slice = attnT[h][:, c * CHUNK:(c + 1) * CHUNK]
                    copy_ps(at_slice, pvps)
                    rcp = sm_pool.tile([1, CHUNK], F32, name=f"rcp_{h}_{c}", tag="rcp")
                    nc.vector.reciprocal(rcp, lps)
                    rb = rb_pool.tile([P, CHUNK], F32, name=f"rb_{h}_{c}", tag="rb")
                    nc.gpsimd.partition_broadcast(rb, rcp)
                    nc.vector.tensor_mul(at_slice, at_slice, rb)

        # ---------------- phase 3: o_proj ----------------
        n_dt = DQ // P
        ecg = min(2, H // 512)        # 512-col groups per psum set
        n_eg = H // (512 * ecg)
        with tc.tile_pool(name="wo_sb", bufs=1) as wo_pool, \
             tc.tile_pool(name="o_ps", bufs=2, space="PSUM") as o_psum, \
             tc.tile_pool(name="o_out", bufs=3) as out_pool:
            for eg in range(n_eg):
                ecols = 512 * ecg
                wo_sb = [wo_pool.tile([P, ecols], F32R, name=f"wo{d}_{eg}",
                                      tag=f"wo{d}") for d in range(n_dt)]
                for d in range(n_dt):
                    nc.gpsimd.dma_start(
                        wo_sb[d],
                        wo[d * P:(d + 1) * P,
                           eg * ecols:(eg + 1) * ecols].bitcast(F32R))
                for tq in range(n_tt):
                    ops = o_psum.tile([P, ecols], F32, name=f"ops_{eg}_{tq}",
                                      tag="ops")
                    for d in range(n_dt):
                        for e in range(ecg):
                            nc.tensor.matmul(
                                ops[:, e * 512:(e + 1) * 512],
                                _r(attnT[d][:, tq * P:(tq + 1) * P]),
                                _r(wo_sb[d][:, e * 512:(e + 1) * 512]),
                                start=(d == 0), stop=(d == n_dt - 1))
                    ot = out_pool.tile([P, ecols], F32, name=f"ot_{eg}_{tq}",
                                       tag="ot")
                    for e in range(ecg):
                        copy_ps(ot[:, e * 512:(e + 1) * 512],
                                ops[:, e * 512:(e + 1) * 512])
                    nc.sync.dma_start(
                        out[tq * P:(tq + 1) * P, eg * ecols:(eg + 1) * ecols], ot)

    nc.compile()
    return nc


def shard_inputs(hidden_states, w_q, w_k, w_v, w_o):
    """Full inputs -> list of 8 per-core input dicts (core i: batch i//TP,
    kv pair i%TP)."""
    ins = []
    for i in range(N_CORES):
        b, p = i // TP, i % TP
        ins.append({
            "hid": np.ascontiguousarray(hidden_states[b]),
            "wq": np.ascontiguousarray(w_q[:, p * 1024:(p + 1) * 1024]),
            "wk": np.ascontiguousarray(w_k[:, p * 256:(p + 1) * 256]),
            "wv": np.ascontiguousarray(w_v[:, p * 256:(p + 1) * 256]),
            "wo": np.ascontiguousarray(w_o[p * 1024:(p + 1) * 1024, :]),
        })
    return ins


def unshard_output(results):
    """8 per-core partial [S,H] outputs -> full [B,S,H]."""
    out = np.zeros((B, S_FULL, H_FULL), dtype=np.float32)
    for i in range(N_CORES):
        out[i // TP] += results[i]["out"]
    return out


_NC_CACHE = {}


def get_nc():
    if "nc" not in _NC_CACHE:
        _NC_CACHE["nc"] = build_attention_nc()
    return _NC_CACHE["nc"]


def kernel(hidden_states, w_q, w_k, w_v, w_o):
    from concourse.bass_utils import run_bass_kernel_spmd
    nc = get_nc()
    ins = shard_inputs(np.asarray(hidden_states, dtype=np.float32),
                       np.asarray(w_q, dtype=np.float32),
                       np.asarray(w_k, dtype=np.float32),
                       np.asarray(w_v, dtype=np.float32),
                       np.asarray(w_o, dtype=np.float32))
    res = run_bass_kernel_spmd(nc, ins, core_ids=list(range(N_CORES)))
    return unshard_output(res.results)


# revision 15
# speedup vs baseline: 1.2024x; 1.2024x over previous
"""GQA causal attention (Llama prefill) on 8 TRN2 NeuronCores.

Sharding: tensor-parallel over KV heads (4-way: 2 KV heads -> 8 Q heads per
core) x data-parallel over batch (2-way).  Core i handles batch i//4 and KV
head pair i%4.  Each core computes its 8 heads' attention and a partial
o_proj ([S, H] contribution from its 1024 columns of attn output); the host
sums the 4 partials per batch element.

Layout strategy (everything transposed so no transposes are needed after the
initial hidden -> hidden^T step):
  hidT[h, t]  (PE transpose of streamed hidden tiles)
  qT[d, t] = w_q^T-slice . hidT   (w_q tile as stationary)
  kT[d, t]   likewise;  v[t, d] natural (hidT tile as stationary)
  S^T[tk, tq] = kT-tile^T . qT    (softmax sums land on matmul-with-ones)
  P^T = exp(S^T * scale) with causal handling per straddle tile
  attnT[d, tq] += v-tile^T . P^T  ;  l[1, tq] += ones^T . P^T
  attnT *= broadcast(1/l)
  out[tq, e] += attnT-tile^T . w_o  (attnT tile as stationary)
Matmuls run as float32r (full-rate fp32 PE mode; fp32 storage, engines
round producer outputs to fp32r).
"""

import numpy as np
from contextlib import ExitStack

import concourse.bass as bass
import concourse.tile as tile
from concourse import mybir, bacc
from concourse.masks import make_identity

F32 = mybir.dt.float32
F32R = mybir.dt.float32r
P = 128

# full-problem config
NUM_HEADS = 32
NUM_KV_HEADS = 8
HEAD_DIM = 128
B, S_FULL, H_FULL = 2, 2048, 4096
TP = 4                     # kv-head-pair shards
DP = 2                     # batch shards
N_CORES = TP * DP


def _r(ap):
    return ap if ap.dtype == F32R else ap.bitcast(F32R)


def build_attention_nc(S=2048, H=4096, n_qh=8, n_kvh=2, D=128, CHUNK=512):
    """One core's program: hid [S,H] x wq [H,n_qh*D] x wk/wv [H,n_kvh*D]
    x wo [n_qh*D, H] -> partial out [S,H]."""
    DQ = n_qh * D
    DKV = n_kvh * D
    g = n_qh // n_kvh
    scale = float(D) ** -0.5
    n_ht = H // P              # h-tiles
    n_ch = S // CHUNK          # token chunks
    spc = CHUNK // P           # 128-subtiles per chunk
    n_tt = S // P              # t-tiles
    W_IN = min(1024, H)        # hidden-load width
    assert S % CHUNK == 0 and CHUNK % P == 0 and H % W_IN == 0 and DKV <= 512

    nc = bacc.Bacc("TRN2", target_bir_lowering=False, debug=False)
    hid = nc.dram_tensor("hid", [S, H], F32, kind="ExternalInput").ap()
    wq = nc.dram_tensor("wq", [H, DQ], F32, kind="ExternalInput").ap()
    wk = nc.dram_tensor("wk", [H, DKV], F32, kind="ExternalInput").ap()
    wv = nc.dram_tensor("wv", [H, DKV], F32, kind="ExternalInput").ap()
    wo = nc.dram_tensor("wo", [DQ, H], F32, kind="ExternalInput").ap()
    out = nc.dram_tensor("out", [S, H], F32, kind="ExternalOutput").ap()

    # alternate psum->sbuf copies between ACT and DVE
    _cp_state = [0]

    def copy_ps(dst, src):
        _cp_state[0] ^= 1
        if _cp_state[0]:
            nc.scalar.copy(dst, src)
        else:
            nc.vector.tensor_copy(dst, src)

    with tile.TileContext(nc) as tc, ExitStack() as ctx:
        const = ctx.enter_context(tc.tile_pool(name="const", bufs=1))
        persist = ctx.enter_context(tc.tile_pool(name="persist", bufs=1))

        ident = const.tile([P, P], F32, name="ident")
        make_identity(nc, ident)
        ones_f32 = const.tile([P, 1], F32, name="ones_f32")
        nc.vector.memset(ones_f32, 1.0)
        ones_col = const.tile([P, 1], F32R, name="ones_col")
        nc.vector.tensor_copy(ones_col, ones_f32)
        zeros_pad = const.tile([P, 512 - P], F32, name="zeros_pad")
        nc.vector.memset(zeros_pad, 0.0)
        # inclusive lower-triangular keep-mask: tri[r, f] = 1.0 iff r <= f
        tri = const.tile([P, P], F32, name="tri")
        nc.vector.memset(tri, 1.0)
        nc.gpsimd.affine_select(
            out=tri, in_=tri, pattern=[[1, P]], base=0, channel_multiplier=-1,
            compare_op=mybir.AluOpType.is_ge, fill=0.0,
        )

        qT = [persist.tile([P, S], F32R, name=f"qT{d}", tag=f"qT{d}")
              for d in range(n_qh)]
        kT = [persist.tile([P, S], F32R, name=f"kT{d}", tag=f"kT{d}")
              for d in range(n_kvh)]
        v_sb = [persist.tile([P, DKV], F32R, name=f"v{j}", tag=f"v{j}")
                for j in range(n_tt)]

        # ---------------- phase 1: hidT + q/k/v projections ----------------
        with tc.tile_pool(name="hidT", bufs=1) as hidT_pool, \
             tc.tile_pool(name="p_in", bufs=2) as in_pool, \
             tc.tile_pool(name="p_w", bufs=3) as w_pool, \
             tc.tile_pool(name="tp_ps", bufs=3, space="PSUM") as tp_psum, \
             tc.tile_pool(name="pp_ps", bufs=4, space="PSUM") as pp_psum:
            for c in range(n_ch):
                with nc.named_scope(f"proj{c}"):
                    # hidT = hidden[c-chunk, :]^T as one [128, n_ht, CHUNK] tile
                    hidT = hidT_pool.tile([P, n_ht, CHUNK], F32R,
                                          name=f"hidT_{c}", tag="hidT")
                    for hc in range(H // W_IN):
                        for ts_ in range(spc):
                            hin = in_pool.tile([P, W_IN], F32,
                                               name=f"hin_{c}_{hc}_{ts_}",
                                               tag="hid_in")
                            nc.scalar.dma_start(
                                hin,
                                hid[c * CHUNK + ts_ * P: c * CHUNK + (ts_ + 1) * P,
                                    hc * W_IN:(hc + 1) * W_IN])
                            for half in range(W_IN // 512):
                                tp = tp_psum.tile([P, 512], F32,
                                                  name=f"tp_{c}_{hc}_{ts_}_{half}",
                                                  tag="tp")
                                for s4 in range(4):
                                    nc.tensor.transpose(
                                        tp[:, s4 * P:(s4 + 1) * P],
                                        hin[:, half * 512 + s4 * P:
                                            half * 512 + (s4 + 1) * P], ident)
                                # grouped copy: 4 h-tiles' columns in one op
                                hi0 = (hc * W_IN + half * 512) // P
                                copy_ps(
                                    hidT[:, hi0:hi0 + 4, ts_ * P:(ts_ + 1) * P],
                                    tp.rearrange("p (s f) -> p s f", s=4))

                    # k^T + v projections in one sweep over h-tiles
                    kps = [pp_psum.tile([P, CHUNK], F32, name=f"kps{d}_{c}",
                                        tag="pp") for d in range(n_kvh)]
                    for hi2 in range(n_ht // 2):
                        wkt = w_pool.tile([P, 2, DKV], F32R,
                                          name=f"wk_{c}_{hi2}", tag="wk_in")
                        nc.gpsimd.dma_start(
                            wkt, wk[hi2 * 2 * P:(hi2 + 1) * 2 * P, :]
                            .rearrange("(s p) d -> p s d", p=P).bitcast(F32R))
                        for s in range(2):
                            hi = hi2 * 2 + s
                            for d in range(n_kvh):
                                nc.tensor.matmul(
                                    kps[d], wkt[:, s, d * P:(d + 1) * P],
                                    hidT[:, hi, :],
                                    start=(hi == 0), stop=(hi == n_ht - 1))
                    for d in range(n_kvh):
                        copy_ps(kT[d][:, c * CHUNK:(c + 1) * CHUNK], kps[d])

                    vps = [pp_psum.tile([P, DKV], F32, name=f"vps{t}_{c}",
                                        tag="pp") for t in range(spc)]
                    for hi2 in range(n_ht // 2):
                        wvt = w_pool.tile([P, 2, DKV], F32R,
                                          name=f"wv_{c}_{hi2}", tag="wv_in")
                        nc.gpsimd.dma_start(
                            wvt, wv[hi2 * 2 * P:(hi2 + 1) * 2 * P, :]
                            .rearrange("(s p) d -> p s d", p=P).bitcast(F32R))
                        for s in range(2):
                            hi = hi2 * 2 + s
                            for t in range(spc):
                                nc.tensor.matmul(
                                    vps[t], hidT[:, hi, t * P:(t + 1) * P],
                                    wvt[:, s, :],
                                    start=(hi == 0), stop=(hi == n_ht - 1))
                    for t in range(spc):
                        copy_ps(v_sb[c * spc + t], vps[t])

                    # q^T projection: 2 sweeps x 4 d-tiles
                    for grp in range(n_qh // 4):
                        qps = [pp_psum.tile([P, CHUNK], F32,
                                            name=f"qps{grp}_{d}_{c}", tag="pp")
                               for d in range(4)]
                        for hi in range(n_ht):
                            wqt = w_pool.tile([P, 4 * P], F32R,
                                              name=f"wq_{c}_{grp}_{hi}",
                                              tag="wq_in")
                            nc.sync.dma_start(
                                wqt, wq[hi * P:(hi + 1) * P,
                                        grp * 4 * P:(grp + 1) * 4 * P]
                                .bitcast(F32R))
                            for d in range(4):
                                nc.tensor.matmul(
                                    qps[d], wqt[:, d * P:(d + 1) * P],
                                    hidT[:, hi, :],
                                    start=(hi == 0), stop=(hi == n_ht - 1))
                        for d in range(4):
                            copy_ps(qT[grp * 4 + d][:, c * CHUNK:(c + 1) * CHUNK],
                                    qps[d])

        # ---------------- phase 2: attention ----------------
        # Per head: chunks paired (a, n_ch-1-a) and their j-loops interleaved
        # so the PE always has an independent S/PV/l chain to run while the
        # other chain's exp/mask round-trips through ACT/DVE.
        # attnT[h] reuses qT[h-1]'s slot (dead once head h-1's scores are
        # done); attnT[0] goes to a fresh pool reusing released hidT space.
        att0_pool = ctx.enter_context(tc.tile_pool(name="att0", bufs=1))
        with tc.tile_pool(name="s_ps", bufs=2, space="PSUM") as s_psum, \
             tc.tile_pool(name="pv_ps", bufs=1, space="PSUM") as pv_psum, \
             tc.tile_pool(name="l_ps", bufs=1, space="PSUM") as l_psum, \
             tc.tile_pool(name="pt", bufs=3) as pt_pool, \
             tc.tile_pool(name="att_sm", bufs=2) as sm_pool, \
             tc.tile_pool(name="att_rb", bufs=2) as rb_pool:
            attnT = []
            for d in range(n_qh):
                if d == 0:
                    attnT.append(att0_pool.tile([P, S], F32R, name="attnT0"))
                else:
                    attnT.append(persist.tile([P, S], F32R, name=f"attnT{d}",
                                              tag=f"qT{d - 1}"))

            if n_ch % 2 == 0:
                pairs = [(a, n_ch - 1 - a) for a in range(n_ch // 2)]
            else:
                pairs = [(c,) for c in range(n_ch)]

            def chain_step(h, kv, ci, c, j):
                jmax = spc * (c + 1)
                sps = s_psum.tile([P, CHUNK], F32, name=f"s_{h}_{c}_{j}",
                                  tag=f"s{ci}")
                nc.tensor.matmul(
                    sps, kT[kv][:, j * P:(j + 1) * P],
                    qT[h][:, c * CHUNK:(c + 1) * CHUNK],
                    start=True, stop=True)
                pt = pt_pool.tile([P, CHUNK], F32R, name=f"pt_{h}_{c}_{j}",
                                  tag=f"pt{ci}")
                p_ = j - spc * c
                if p_ < 0:
                    # full block
                    nc.scalar.activation(pt, sps,
                                         mybir.ActivationFunctionType.Exp,
                                         scale=scale)
                else:
                    # straddle: cols < 128p fully masked; diag block needs tri
                    if p_ > 0:
                        nc.vector.tensor_copy(pt[:, :p_ * P],
                                              zeros_pad[:, :p_ * P])
                    nc.scalar.activation(pt[:, p_ * P:], sps[:, p_ * P:],
                                         mybir.ActivationFunctionType.Exp,
                                         scale=scale)
                    nc.vector.tensor_mul(pt[:, p_ * P:(p_ + 1) * P],
                                         pt[:, p_ * P:(p_ + 1) * P], tri)
                return pt

            for h in range(n_qh):
                kv = h // g
                with nc.named_scope(f"attn{h}"):
                    for pair in pairs:
                        chains = []
                        for ci, c in enumerate(pair):
                            jmax = spc * (c + 1)
                            pvps = pv_psum.tile([P, CHUNK], F32,
                                                name=f"pv_{h}_{c}", tag=f"pv{ci}")
                            lps = l_psum.tile([1, CHUNK], F32,
                                              name=f"l_{h}_{c}", tag=f"l{ci}")
                            chains.append({"c": c, "ci": ci, "jmax": jmax,
                                           "pv": pvps, "l": lps})
                        nsteps = max(ch["jmax"] for ch in chains)
                        for j in range(nsteps):
                            for ch in chains:
                                if j >= ch["jmax"]:
                                    continue
                                c = ch["c"]
                                pt = chain_step(h, kv, ch["ci"], c, j)
                                nc.tensor.matmul(
                                    ch["pv"], v_sb[j][:, kv * P:(kv + 1) * P],
                                    pt, start=(j == 0), stop=(j == ch["jmax"] - 1))
                                nc.tensor.matmul(
                                    ch["l"], ones_col, pt,
                                    start=(j == 0), stop=(j == ch["jmax"] - 1))
                        for ch in chains:
                            c = ch["c"]
                            at_slice = attnT[h][:, c * CHUNK:(c + 1) * CHUNK]
                            copy_ps(at_slice, ch["pv"])
                            rcp = sm_pool.tile([1, CHUNK], F32,
                                               name=f"rcp_{h}_{c}", tag="rcp")
                            nc.vector.reciprocal(rcp, ch["l"])
                            rb = rb_pool.tile([P, CHUNK], F32,
                                              name=f"rb_{h}_{c}", tag="rb")
                            nc.gpsimd.partition_broadcast(rb, rcp)
                            nc.vector.tensor_mul(at_slice, at_slice, rb)

        # ---------------- phase 3: o_proj ----------------
        n_dt = DQ // P
        ecg = min(2, H // 512)        # 512-col groups per psum set
        n_eg = H // (512 * ecg)
        with tc.tile_pool(name="wo_sb", bufs=1) as wo_pool, \
             tc.tile_pool(name="o_ps", bufs=3, space="PSUM") as o_psum, \
             tc.tile_pool(name="o_out", bufs=3) as out_pool:
            for eg in range(n_eg):
                with nc.named_scope(f"oproj{eg}"):
                    ecols = 512 * ecg
                    wo_sb = [wo_pool.tile([P, ecols], F32R, name=f"wo{d}_{eg}",
                                          tag=f"wo{d}") for d in range(n_dt)]
                    for d in range(n_dt):
                        nc.gpsimd.dma_start(
                            wo_sb[d],
                            wo[d * P:(d + 1) * P,
                               eg * ecols:(eg + 1) * ecols].bitcast(F32R))
                    for tq in range(n_tt):
                        ops = o_psum.tile([P, ecols], F32, name=f"ops_{eg}_{tq}",
                                          tag="ops")
                        for d in range(n_dt):
                            for e in range(ecg):
                                nc.tensor.matmul(
                                    ops[:, e * 512:(e + 1) * 512],
                                    attnT[d][:, tq * P:(tq + 1) * P],
                                    wo_sb[d][:, e * 512:(e + 1) * 512],
                                    start=(d == 0), stop=(d == n_dt - 1))
                        ot = out_pool.tile([P, ecols], F32, name=f"ot_{eg}_{tq}",
                                           tag="ot")
                        copy_ps(ot, ops)
                        nc.sync.dma_start(
                            out[tq * P:(tq + 1) * P,
                                eg * ecols:(eg + 1) * ecols], ot)

    nc.compile()
    return nc


def shard_inputs(hidden_states, w_q, w_k, w_v, w_o):
    """Full inputs -> list of 8 per-core input dicts (core i: batch i//TP,
    kv pair i%TP)."""
    ins = []
    for i in range(N_CORES):
        b, p = i // TP, i % TP
        ins.append({
            "hid": np.ascontiguousarray(hidden_states[b]),
            "wq": np.ascontiguousarray(w_q[:, p * 1024:(p + 1) * 1024]),
            "wk": np.ascontiguousarray(w_k[:, p * 256:(p + 1) * 256]),
            "wv": np.ascontiguousarray(w_v[:, p * 256:(p + 1) * 256]),
            "wo": np.ascontiguousarray(w_o[p * 1024:(p + 1) * 1024, :]),
        })
    return ins


def unshard_output(results):
    """8 per-core partial [S,H] outputs -> full [B,S,H]."""
    out = np.zeros((B, S_FULL, H_FULL), dtype=np.float32)
    for i in range(N_CORES):
        out[i // TP] += results[i]["out"]
    return out


_NC_CACHE = {}


def get_nc():
    if "nc" not in _NC_CACHE:
        _NC_CACHE["nc"] = build_attention_nc()
    return _NC_CACHE["nc"]


def kernel(hidden_states, w_q, w_k, w_v, w_o):
    from concourse.bass_utils import run_bass_kernel_spmd
    nc = get_nc()
    ins = shard_inputs(np.asarray(hidden_states, dtype=np.float32),
                       np.asarray(w_q, dtype=np.float32),
                       np.asarray(w_k, dtype=np.float32),
                       np.asarray(w_v, dtype=np.float32),
                       np.asarray(w_o, dtype=np.float32))
    res = run_bass_kernel_spmd(nc, ins, core_ids=list(range(N_CORES)))
    return unshard_output(res.results)
